# revision 13
# baseline (speedup 1.0000x reference)
"""CDVAE encoder GNN — Trainium2 Bass kernel (8-core data-parallel over graphs).

Structure (same math as the validated baseline, rescheduled):
 - coords pathway dead for (mu, logvar); dist/features computed once.
 - Dense per-graph 32x32 edge pass: h1 = [Wc'; a_g; b_g]^T @ [sinfeat; di; dj]
   (edge_lin folded into Wc' host-side; diagonal columns zeroed).
 - j-sum commutes with edge_w2: nm = (sum_j silu(h1)) @ W2 + 31*b2.

Scheduling rewrite vs baseline (815us):
 - Flat software pipeline over (layer, 4-graph slot): emit C(prev slot),
   A(slot), B(slot).  Tensor runs a slot ahead; ACT (the silu floor,
   ~76us/layer) is never starved.
 - All bias adds via ones-row matmuls (off the ACT engine); PSUM->SBUF
   moves on DVE.
 - ssub in bf16 so the DVE j-reduce hits the 2x packed mode (the f32 out
   forced 1x = 2.24us/tile in the baseline); cast-up sliver keeps the nm
   matmul in f32r.
 - eac/wcab/staging in bf16 (SBUF 169KB, halved edge DMA).
 - PSUM: 3x[128,1024] B-tiles + 2x[128,512] A/C tiles = 8 banks.
 - sin-feature units interleaved into the 4 layer-0 slots.

Sharding: 16 graphs per core, weights replicated, no collectives.
"""

import math
import numpy as np

import concourse.bass as bass
import concourse.mybir as mybir
import concourse.tile as tile
from concourse import bacc
from concourse.bass import ds, ts
from concourse.masks import make_identity

F32 = mybir.dt.float32
F32R = mybir.dt.float32r
BF16 = mybir.dt.bfloat16
I32 = mybir.dt.int32
AF = mybir.ActivationFunctionType
ALU = mybir.AluOpType

G_TOT = 128      # graphs total
NA = 32          # atoms per graph
GPC = 16         # graphs per core
NPC = GPC * NA   # nodes per core (512)
H = 512
ED = 64
L = 6
LAT2 = 512       # 2*latent
NCORES = 8

TWO_PI = 2.0 * math.pi
RNE_MAGIC = 1.5 * 2.0 ** 23          # fp32 round-to-nearest-int trick


def _r(ap):
    """bitcast an fp32 AP to float32r for full-rate matmul."""
    return ap.bitcast(F32R)


def build_module():
    """Build the per-core Bass module (same program on all 8 cores)."""
    nc = bacc.Bacc("TRN2", target_bir_lowering=False, debug=False)

    # ---- DRAM tensors (per-core inputs) ----
    def din(name, shape, dtype=F32):
        return nc.dram_tensor(name, list(shape), dtype, kind="ExternalInput").ap()

    coords_t = din("coords_t", (3, NPC))
    atypes = din("atypes", (NPC,), I32)
    aembed = din("aembed", (100, H))
    econst = din("econst", (ED, NA * NA), BF16)
    qs_pack = din("qs_pack", (4, ED))
    wa = din("wa", (L, H, H))
    wb = din("wb", (L, H, H))
    wcp = din("wcp", (L, ED, H), BF16)
    wn1a = din("wn1a", (L, H, H))
    w2w1b = din("w2w1b", (L, H, H))
    wn2 = din("wn2", (L, H, H))
    brows = din("brows", (L, 1, 4 * H), BF16)      # concat: b1', 31*b2, nb1, nb2
    gw1 = din("gw1", (H, H))                 # pre-divided by NA
    gw2 = din("gw2", (H, LAT2))
    gbrow = din("gbrow", (1, 2 * LAT2), BF16)      # concat: gb1, gb2

    lat_out = nc.dram_tensor("lat", [LAT2, GPC], F32, kind="ExternalOutput").ap()
    if DEBUG_DUMPS:
        d_eac = nc.dram_tensor(
            "d_eac", [128, GPC * NA * NA], BF16, kind="ExternalOutput").ap()
        d_wcab = nc.dram_tensor(
            "d_wcab", [128, GPC * H], BF16, kind="ExternalOutput").ap()
        d_ssub = nc.dram_tensor(
            "d_ssub", [128, 4 * H], BF16, kind="ExternalOutput").ap()
        d_node1 = nc.dram_tensor(
            "d_node1", [128, 4 * H], F32, kind="ExternalOutput").ap()

    with tile.TileContext(nc) as tc:
        with (
            tc.tile_pool(name="cpool", bufs=1) as cpool,
            tc.tile_pool(name="npool", bufs=2) as npool,
            tc.tile_pool(name="wabp", bufs=2) as wabp,
            tc.tile_pool(name="wcabp", bufs=2) as wcabp,
            tc.tile_pool(name="wsm", bufs=2) as wsm,
            tc.tile_pool(name="sgp", bufs=3) as sgp,
            tc.tile_pool(name="stg", bufs=2) as stg,
            tc.tile_pool(name="z1gp", bufs=2) as z1gp,
            tc.tile_pool(name="z1hp", bufs=2) as z1hp,
            tc.tile_pool(name="kdfp", bufs=2) as kdfp,
            tc.tile_pool(name="ssfp", bufs=2) as ssfp,
            tc.tile_pool(name="pb", bufs=3, space="PSUM") as pb,
            tc.tile_pool(name="pac", bufs=2, space="PSUM") as pac,
        ):
            # ---------- persistent tiles ----------
            eac_sb = cpool.tile([128, GPC * NA * NA], BF16, tag="eac")  # 32 KB/p
            qs_sb = cpool.tile([4, ED], F32, tag="qs")
            ct_sb = cpool.tile([3, NPC], F32, tag="ct")
            ones3 = cpool.tile([3, NA], F32, tag="ones3")
            ident = cpool.tile([128, 128], F32, tag="ident")

            ssub = cpool.tile([128, 4, H], BF16, tag="ssub")
            graph_t = cpool.tile([128, 4, GPC], F32, tag="graph")
            q1s = cpool.tile([128, 4, GPC], F32, tag="q1s")
            lat_sb = cpool.tile([128, 4, GPC], F32, tag="lat")
            dscA = cpool.tile([4, NA * NA], F32, tag="dscA")
            dscB = cpool.tile([4, NA * NA], F32, tag="dscB")
            dhpp = cpool.tile([128, 4 * NA], F32, tag="dhpp")
            dlpp = cpool.tile([128, 4 * NA], F32, tag="dlpp")
            gbrow_t = cpool.tile([1, 2 * LAT2], BF16, tag="gbrow")
            ones1b = cpool.tile([1, 128], BF16, tag="ones1b")
            # per-layer re-DMA'd weight singles (C phase)
            wn1a_t = cpool.tile([128, 4, H], F32, tag="wn1at")
            w2w1b_t = cpool.tile([128, 4, H], F32, tag="w2w1bt")
            wn2_t = cpool.tile([128, 4, H], F32, tag="wn2t")

            _mark(nc, "setup")
            # ---------- prelude ----------
            node0 = npool.tile([128, 4, H], F32, tag="node")
            with (
                tc.tile_pool(name="spool", bufs=1) as spool,
                tc.tile_pool(name="sp2", bufs=2) as sp2,
            ):
                at_sb = spool.tile([128, 4], I32, tag="at")
                nr_tiles = []
                # node gather FIRST (before any large DMAs: a big broadcast
                # in flight breaks the gather's offset read)
                for t in range(4):
                    nc.sync.dma_start(
                        out=at_sb[:, t : t + 1],
                        in_=atypes[ds(t * 128, 128)].rearrange("(p o) -> p o", o=1),
                    )
                for t in range(4):
                    nr = sp2.tile([128, H], F32, tag="nr", name=f"nr{t}")
                    nc.gpsimd.indirect_dma_start(
                        out=nr[:],
                        out_offset=None,
                        in_=aembed,
                        in_offset=bass.IndirectOffsetOnAxis(
                            ap=at_sb[:, t : t + 1], axis=0
                        ),
                    )
                    nr_tiles.append(nr)
                # delta_i / delta_j indicator rows (diag pre-zeroed host-side)
                nc.sync.dma_start(
                    out=eac_sb[ED:128, :].rearrange("p (g e) -> p g e", e=NA * NA),
                    in_=econst.unsqueeze(1).broadcast_to([ED, GPC, NA * NA]),
                )
                emit_wcab_bcast(0)
                nc.sync.dma_start(out=_r(qs_sb[:]), in_=_r(qs_pack))
                nc.sync.dma_start(out=ct_sb[:], in_=coords_t)
                nc.sync.dma_start(out=gbrow_t[:], in_=gbrow)
                nc.vector.memset(ones3[:], 1.0)
                nc.vector.memset(ones1b[:], 1.0)
                nc.vector.memset(dscA[:], 1.0)
                nc.vector.memset(dscB[:], 1.0)
                make_identity(nc, ident[:])

                # layer-0 weights (A needs them at slot (0,0))
                wab_t0 = wabp.tile([128, 8, H], F32, tag="wab")
                nc.sync.dma_start(
                    out=_r(wab_t0[:, 0:4, :]),
                    in_=_r(wa[0].rearrange("(c p) m -> p c m", p=128)),
                )
                nc.sync.dma_start(
                    out=_r(wab_t0[:, 4:8, :]),
                    in_=_r(wb[0].rearrange("(c p) m -> p c m", p=128)),
                )
                brows_t0 = wsm.tile([1, 4 * H], BF16, tag="brows")
                nc.sync.dma_start(out=brows_t0[:], in_=brows[0])

                # transpose 16 128x128 blocks: node0[hc*128+h, t*128+n] = node[n, h]
                for t in range(4):
                    pt = pac.tile([128, 512], F32, tag="pac")
                    for kc in range(4):
                        nc.tensor.transpose(
                            out=pt[:, ds(kc * 128, 128)],
                            in_=nr_tiles[t][:, ds(kc * 128, 128)],
                            identity=ident[:],
                        )
                    nc.vector.tensor_copy(
                        out=_r(node0[:, :, ds(t * 128, 128)]),
                        in_=pt[:].rearrange("p (kc f) -> p kc f", f=128),
                    )

                # ---------- distances ----------
                # d2pp[gg*32+i, q*32+j] = |c_i - c_j|^2 of graph g = q*4+gg
                ctm2 = spool.tile([3, NPC], F32, tag="ctm2")
                ctsq = spool.tile([3, NPC], F32, tag="ctsq")
                nc.vector.tensor_scalar(
                    out=ctm2[:], in0=ct_sb[:], scalar1=-2.0, scalar2=None,
                    op0=ALU.mult,
                )
                nc.vector.tensor_tensor(
                    out=ctsq[:], in0=ct_sb[:], in1=ct_sb[:], op=ALU.mult
                )
                d2pp = spool.tile([128, 4 * NA], F32, tag="d2pp")
                for q in range(4):
                    pd = pac.tile([128, 512], F32, tag="pac")
                    for gg in range(4):
                        g = q * 4 + gg
                        csl = ds(g * NA, NA)
                        tp = (0, 32 * gg) if gg else None
                        osl = pd[ds(32 * gg, 32), 0:NA]
                        nc.tensor.matmul(
                            out=osl, lhsT=ct_sb[:, csl], rhs=ctm2[:, csl],
                            start=True, stop=False, tile_position=tp,
                        )
                        nc.tensor.matmul(
                            out=osl, lhsT=ctsq[:, csl], rhs=ones3[:],
                            start=False, stop=False, tile_position=tp,
                        )
                        nc.tensor.matmul(
                            out=osl, lhsT=ones3[:], rhs=ctsq[:, csl],
                            start=False, stop=True, tile_position=tp,
                        )
                    nc.vector.tensor_scalar(
                        out=d2pp[:, ds(q * NA, NA)], in0=pd[:, 0:NA], scalar1=1e-12,
                        scalar2=None, op0=ALU.max,
                    )
                s0 = spool.tile([128, 4 * NA], F32, tag="s0")
                nc.scalar.activation(s0[:], d2pp[:], AF.Sqrt)
                rr = spool.tile([128, 4 * NA], F32, tag="rr")
                nc.vector.reciprocal(out=rr[:], in_=s0[:])
                t1 = spool.tile([128, 4 * NA], F32, tag="t1")
                nc.vector.tensor_tensor(
                    out=t1[:], in0=d2pp[:], in1=rr[:], op=ALU.mult
                )
                # dsum = s0 + d2/s0 = 2*dist (Newton); the x0.5 is in qs_pack
                dspp = spool.tile([128, 4 * NA], F32, tag="dspp")
                nc.vector.tensor_tensor(
                    out=dspp[:], in0=s0[:], in1=t1[:], op=ALU.add
                )
                # hi/lo mantissa split so f32r products stay (near-)exact
                maskc = spool.tile([128, 1], I32, tag="maskc")
                nc.vector.memset(maskc[:], -4096)     # 0xFFFFF000
                nc.vector.tensor_scalar(
                    out=dhpp[:].bitcast(I32), in0=dspp[:].bitcast(I32),
                    scalar1=maskc[:], scalar2=None,
                    op0=ALU.bitwise_and,
                )
                nc.vector.tensor_tensor(
                    out=dlpp[:], in0=dspp[:], in1=dhpp[:], op=ALU.subtract
                )

            # ---------- pipeline helpers ----------
            wab_tiles = {0: wab_t0}
            wcab_tiles = {}
            brows_tiles = {0: brows_t0}
            node_tiles = {0: node0}

            def sin_unit(g):
                """sin/cos features for graph g -> eac rows 0:64."""
                psl = ds((g % 4) * NA, NA)
                fsl = ds((g // 4) * NA, NA)
                dsc = dscA if g % 2 == 0 else dscB
                nc.sync.dma_start(out=_r(dsc[0:1, :]), in_=_r(dhpp[psl, fsl]))
                nc.sync.dma_start(out=_r(dsc[1:2, :]), in_=_r(dlpp[psl, fsl]))
                nc.sync.dma_start(out=_r(dsc[2:3, :]), in_=_r(dhpp[psl, fsl]))
                ptq = pb.tile([128, NA * NA], F32, tag="pb")
                for s in range(2):
                    nsl = ds(s * 512, 512)
                    nc.tensor.matmul(
                        out=ptq[0:ED, nsl], lhsT=_r(qs_sb[:]),
                        rhs=_r(dsc[:, nsl]), start=True, stop=True,
                    )
                kk = kdfp.tile([ED, NA * NA], F32, tag="kdf")
                nc.vector.tensor_scalar(
                    out=kk[:], in0=ptq[0:ED, :],
                    scalar1=RNE_MAGIC, scalar2=RNE_MAGIC,
                    op0=ALU.add, op1=ALU.subtract,
                )
                df = kdfp.tile([ED, NA * NA], F32, tag="kdf")
                nc.vector.scalar_tensor_tensor(
                    out=df[:], in0=kk[:], scalar=-1.0,
                    in1=ptq[0:ED, :], op0=ALU.mult, op1=ALU.add,
                )
                with nc.allow_low_precision(reason="bf16 edge features"):
                    nc.scalar.activation(
                        eac_sb[0:ED, ds(g * NA * NA, NA * NA)],
                        df[:], AF.Sin, scale=TWO_PI,
                    )
                # zero diagonal (i==j) columns: silu(0) = 0 drops out of j-sum
                nc.vector.memset(
                    eac_sb[0:ED, g * NA * NA : (g + 1) * NA * NA : NA + 1], 0.0
                )

            def emit_wC(l):
                """layer-l C-phase weights (AFTER C(l-1,3) consumed layer l-1)."""
                nc.sync.dma_start(
                    out=_r(wn1a_t[:]),
                    in_=_r(wn1a[l].rearrange("(c p) m -> p c m", p=128)),
                )
                nc.sync.dma_start(
                    out=_r(w2w1b_t[:]),
                    in_=_r(w2w1b[l].rearrange("(c p) m -> p c m", p=128)),
                )
                nc.sync.dma_start(
                    out=_r(wn2_t[:]), in_=_r(wn2[l].rearrange("(c p) m -> p c m", p=128))
                )

            def emit_wcab_bcast(l):
                # Wc' -> wcab rows 0:64 (replicated per graph)
                wt = wcabp.tile([128, GPC, H], BF16, tag="wcab", name=f"wcab{l}")
                wcab_tiles[l] = wt
                nc.sync.dma_start(
                    out=wt[0:ED, :, :],
                    in_=wcp[l].unsqueeze(1).broadcast_to([ED, GPC, H]),
                )

            def emit_wAB(l):
                wab_t = wabp.tile([128, 8, H], F32, tag="wab")
                nc.sync.dma_start(
                    out=_r(wab_t[:, 0:4, :]),
                    in_=_r(wa[l].rearrange("(c p) m -> p c m", p=128)),
                )
                nc.sync.dma_start(
                    out=_r(wab_t[:, 4:8, :]),
                    in_=_r(wb[l].rearrange("(c p) m -> p c m", p=128)),
                )
                wab_tiles[l] = wab_t
                brows_t = wsm.tile([1, 4 * H], BF16, tag="brows")
                nc.sync.dma_start(out=brows_t[:], in_=brows[l])
                brows_tiles[l] = brows_t

            def emit_A(l, t):
                """a/b projections for graphs 4t..4t+3 -> wcab rows 64:128."""
                node = node_tiles[l]
                wab_t = wab_tiles[l]
                brows_t = brows_tiles[l]
                tsl = ds(t * 128, 128)
                pab = pac.tile([128, 512], F32, tag="pac")
                for kc in range(4):
                    nc.tensor.matmul(
                        out=pab[:], lhsT=_r(node[:, kc, tsl]),
                        rhs=_r(wab_t[:, kc, :]), start=(kc == 0), stop=False,
                    )
                nc.tensor.matmul(
                    out=pab[:], lhsT=ones1b[:], rhs=brows_t[0:1, 0:H],
                    start=False, stop=True,
                )
                sta = stg.tile([128, H], BF16, tag="stg")
                with nc.allow_low_precision(reason="bf16 edge operands"):
                    nc.vector.tensor_copy(out=sta[:], in_=pab[:])
                pbt = pac.tile([128, 512], F32, tag="pac")
                for kc in range(4):
                    nc.tensor.matmul(
                        out=pbt[:], lhsT=_r(node[:, kc, tsl]),
                        rhs=_r(wab_t[:, 4 + kc, :]), start=(kc == 0), stop=(kc == 3),
                    )
                stb = stg.tile([128, H], BF16, tag="stg")
                with nc.allow_low_precision(reason="bf16 edge operands"):
                    nc.vector.tensor_copy(out=stb[:], in_=pbt[:])
                wt = wcab_tiles[l]
                for gg in range(4):
                    g = t * 4 + gg
                    nc.sync.dma_start(
                        out=wt[ED : ED + NA, g, :], in_=sta[ds(gg * NA, NA), :]
                    )
                    nc.sync.dma_start(
                        out=wt[ED + NA : 128, g, :], in_=stb[ds(gg * NA, NA), :]
                    )

            def emit_B(l, t):
                """dense edge pass for graphs 4t..4t+3 -> ssub (bf16)."""
                wt = wcab_tiles[l]
                for hc in range(4):
                    hsl = ds(hc * 128, 128)
                    for gp in range(2):
                        sg = sgp.tile([128, 2 * NA * NA], BF16, tag="sg")
                        for gg in range(2):
                            g = t * 4 + 2 * gp + gg
                            ph = pb.tile([128, NA * NA], F32, tag="pb")
                            for s in range(2):
                                nc.tensor.matmul(
                                    out=ph[:, ds(s * 512, 512)],
                                    lhsT=wt[:, g, hsl],
                                    rhs=eac_sb[:, ds(g * 1024 + s * 512, 512)],
                                    start=True, stop=True,
                                )
                            nc.scalar.activation(
                                sg[:, ds(gg * NA * NA, NA * NA)], ph[:], AF.Silu
                            )
                        with nc.allow_low_precision(reason="bf16 ssub"):
                            nc.gpsimd.pool(
                                out=ssub[:, hc, ds((t * 4 + 2 * gp) * NA, 2 * NA)],
                                in_=sg[:].rearrange("p (i j) -> p i j", j=NA),
                                func=mybir.PoolFunctionType.avg,
                            )

            def emit_C(l, t):
                """node MLP for graphs 4t..4t+3; writes node_{l+1} slice.

                nm is eliminated algebraically: z1 = silu(node@Wn1a +
                ssub@(W2@Wn1b) + b'); all matmuls are data-stationary
                (N=512 streams); node-major results transposed back.
                """
                node = node_tiles[l]
                brows_t = brows_tiles[l]
                tsl = ds(t * 128, 128)
                if l + 1 not in node_tiles:
                    node_tiles[l + 1] = npool.tile(
                        [128, 4, H], F32, tag="node", name=f"node{l + 1}"
                    )
                node_nx = node_tiles[l + 1]
                # cast-up sliver: ssub slice back to f32 for f32r matmuls
                ssf = ssfp.tile([128, 4, 128], F32, tag="ssf")
                nc.vector.tensor_scalar(
                    out=_r(ssf[:]), in0=ssub[:, :, tsl], scalar1=float(NA),
                    scalar2=None, op0=ALU.mult,
                )
                pz = pac.tile([128, 512], F32, tag="pac")
                for kc in range(4):
                    nc.tensor.matmul(
                        out=pz[:], lhsT=_r(node[:, kc, tsl]),
                        rhs=_r(wn1a_t[:, kc, :]), start=(kc == 0), stop=False,
                    )
                for kc in range(4):
                    nc.tensor.matmul(
                        out=pz[:], lhsT=_r(ssf[:, kc, :]),
                        rhs=_r(w2w1b_t[:, kc, :]), start=False, stop=False,
                    )
                nc.tensor.matmul(
                    out=pz[:], lhsT=ones1b[:], rhs=brows_t[0:1, ds(2 * H, H)],
                    start=False, stop=True,
                )
                z1n = z1gp.tile([128, 512], F32, tag="z1n")   # node-major
                nc.scalar.activation(_r(z1n[:]), pz[:], AF.Silu)
                ptz = pac.tile([128, 512], F32, tag="pac")
                for hc in range(4):
                    nc.tensor.transpose(
                        out=ptz[:, ds(hc * 128, 128)],
                        in_=z1n[:, ds(hc * 128, 128)], identity=ident[:],
                    )
                z1h = z1hp.tile([128, 4, 128], F32, tag="z1h")
                nc.vector.tensor_copy(
                    out=_r(z1h[:]), in_=ptz[:].rearrange("p (c m) -> p c m", m=128)
                )
                pz2 = pac.tile([128, 512], F32, tag="pac")
                for kc in range(4):
                    nc.tensor.matmul(
                        out=pz2[:], lhsT=_r(z1h[:, kc, :]),
                        rhs=_r(wn2_t[:, kc, :]), start=(kc == 0), stop=False,
                    )
                nc.tensor.matmul(
                    out=pz2[:], lhsT=ones1b[:], rhs=brows_t[0:1, ds(3 * H, H)],
                    start=False, stop=True,
                )
                nns = z1gp.tile([128, 512], F32, tag="nns")   # node-major
                nc.vector.tensor_copy(out=_r(nns[:]), in_=pz2[:])
                pt2 = pac.tile([128, 512], F32, tag="pac")
                for hc in range(4):
                    nc.tensor.transpose(
                        out=pt2[:, ds(hc * 128, 128)],
                        in_=nns[:, ds(hc * 128, 128)], identity=ident[:],
                    )
                nc.vector.tensor_copy(
                    out=_r(node_nx[:, :, tsl]),
                    in_=pt2[:].rearrange("p (c m) -> p c m", m=128),
                )
                if l == L - 1:
                    # graph pooling (mean folded into gw1)
                    with nc.allow_low_precision(reason="f32r round on write"):
                        nc.vector.tensor_reduce(
                            out=_r(graph_t[:, :, ds(t * 4, 4)]),
                            in_=node_nx[:, :, tsl].rearrange(
                                "p c (g a) -> p c g a", a=NA
                            ),
                            op=ALU.add, axis=mybir.AxisListType.X,
                        )

            # ---------- the pipeline ----------
            SLOTS = [(l, t) for l in range(L) for t in range(4)]
            for k, (l, t) in enumerate(SLOTS):
                _mark(nc, f"S{l}.{t}")
                if DEBUG_DUMPS and (l, t) == (1, 0):
                    nc.sync.dma_start(out=d_eac, in_=eac_sb[:])
                    nc.sync.dma_start(
                        out=d_wcab.rearrange("p (g m) -> p g m", g=GPC), in_=wcab[:])
                    nc.sync.dma_start(
                        out=d_ssub.rearrange("p (c m) -> p c m", c=4), in_=ssub[:])
                if DEBUG_DUMPS and (l, t) == (1, 1):
                    nc.sync.dma_start(
                        out=d_node1.rearrange("p (c m) -> p c m", c=4),
                        in_=node_tiles[1][:])
                if l == 0:
                    for g in range(4 * t, 4 * t + 4):
                        sin_unit(g)
                if t == 2:
                    if l + 1 < L:
                        emit_wAB(l + 1)
                        emit_wcab_bcast(l + 1)
                    else:
                        gw_t = wabp.tile([128, 8, H], F32, tag="wab")
                        nc.sync.dma_start(
                            out=_r(gw_t[:, 0:4, :]),
                            in_=_r(gw1.rearrange("(c p) m -> p c m", p=128)),
                        )
                        nc.sync.dma_start(
                            out=_r(gw_t[:, 4:8, :]),
                            in_=_r(gw2.rearrange("(c p) m -> p c m", p=128)),
                        )
                emit_A(l, t)
                emit_B(l, t)
                if k > 0:
                    lp, tp = SLOTS[k - 1]
                    emit_C(lp, tp)
                if t == 0:
                    emit_wC(l)

            _mark(nc, "final")
            emit_C(L - 1, 3)
            # ---------- final: latent MLP on pooled graphs ----------
            pq = pac.tile([128, 512], F32, tag="pac")
            for hc in range(4):
                osl = pq[:, ds(hc * GPC, GPC)]
                hsl = ds(hc * 128, 128)
                for kc in range(4):
                    nc.tensor.matmul(
                        out=osl, lhsT=_r(gw_t[:, kc, hsl]), rhs=_r(graph_t[:, kc, :]),
                        start=(kc == 0), stop=False,
                    )
                nc.tensor.matmul(
                    out=osl, lhsT=gbrow_t[0:1, ds(hc * 128, 128)], rhs=ones1b[:, 0:GPC],
                    start=False, stop=True,
                )
            nc.scalar.activation(
                _r(q1s[:]), pq[:, 0 : 4 * GPC].rearrange("p (c m) -> p c m", m=GPC),
                AF.Silu,
            )
            pl = pac.tile([128, 512], F32, tag="pac")
            for oc in range(4):
                osl = pl[:, ds(oc * GPC, GPC)]
                for kc in range(4):
                    nc.tensor.matmul(
                        out=osl, lhsT=_r(gw_t[:, 4 + kc, ds(oc * 128, 128)]),
                        rhs=_r(q1s[:, kc, :]), start=(kc == 0), stop=False,
                    )
                nc.tensor.matmul(
                    out=osl, lhsT=gbrow_t[0:1, ds(LAT2 + oc * 128, 128)],
                    rhs=ones1b[:, 0:GPC], start=False, stop=True,
                )
            nc.vector.tensor_copy(
                out=lat_sb[:],
                in_=pl[:, 0 : 4 * GPC].rearrange("p (c m) -> p c m", m=GPC),
            )
            nc.sync.dma_start(
                out=lat_out.rearrange("(c p) g -> p c g", p=128), in_=lat_sb[:]
            )

    nc.compile()
    return nc


def prep_inputs(inputs):
    """Host-side packing: shard per core + weight layout transforms."""
    import ml_dtypes

    f32 = np.float32
    bf16 = ml_dtypes.bfloat16
    coords = np.asarray(inputs["coords"], f32)
    atom_types = np.asarray(inputs["atom_types"], np.int32)
    ew1 = np.asarray(inputs["edge_w1"], f32)
    eb1 = np.asarray(inputs["edge_b1"], f32)
    elinw = np.asarray(inputs["edge_lin_w"], f32)
    elinb = np.asarray(inputs["edge_lin_b"], f32)
    wc_raw = np.ascontiguousarray(ew1[:, 2 * H :, :])          # (L, 64, 512)

    # fold edge_lin into Wc: c = feat @ (elinw @ Wc) + elinb @ Wc
    wcp = np.einsum("fe,leh->lfh", elinw, wc_raw)
    b1p = eb1 + np.einsum("e,leh->lh", elinb, wc_raw)          # (L, 512)

    # indicator matrix: rows 0-31 delta(i), rows 32-63 delta(j); diag zeroed
    ec = np.zeros((ED, NA * NA), f32)
    ii, jj = np.meshgrid(np.arange(NA), np.arange(NA), indexing="ij")
    ii, jj = ii.ravel(), jj.ravel()
    m = ii != jj
    ec[ii[m], np.arange(NA * NA)[m]] = 1.0
    ec[32 + jj[m], np.arange(NA * NA)[m]] = 1.0

    gfp = np.asarray(inputs["gfp_W"], f32)
    # tq = dist*f (+0.25 turn on cos rows) as an exact f32r matmul via
    # hi/lo mantissa splits: rhs rows are (d_hi, d_lo, d_hi, ones), so
    # lhsT rows must be (f_hi, f_hi, f_lo, qshift).
    fq = np.concatenate([gfp, gfp]) * np.float32(0.5)
    f_hi = (fq.view(np.uint32) & np.uint32(0xFFFFF000)).view(np.float32)
    f_lo = (fq - f_hi).astype(f32)
    qs_pack = np.stack([
        f_hi, f_hi, f_lo,
        np.concatenate([np.zeros(32, f32), np.full(32, 0.25, f32)]),
    ]).astype(f32)

    w2_64 = np.asarray(inputs["edge_w2"], np.float64)
    w1b_64 = np.asarray(inputs["node_w1"], np.float64)[:, H:, :]
    w2w1b = np.einsum("lij,ljk->lik", w2_64, w1b_64).astype(f32)
    b2n = np.asarray(inputs["edge_b2"], np.float64) * (NA - 1)
    z1bias = (np.asarray(inputs["node_b1"], np.float64)
              + np.einsum("lj,ljk->lk", b2n, w1b_64)).astype(f32)
    brows = np.stack([
        b1p,                                               # A bias (b1')
        np.zeros_like(b1p),                                # unused
        z1bias,                                            # z1 bias (nm folded)
        np.asarray(inputs["node_b2"], f32),                # node bias
    ], axis=1).reshape(L, 1, 4 * H).astype(bf16)           # (L, 1, 4*512)
    gbrow = np.concatenate([
        np.asarray(inputs["graph_b1"], f32),
        np.asarray(inputs["graph_b2"], f32),
    ]).reshape(1, 2 * LAT2).astype(bf16)                   # (1, 1024)

    shared = {
        "aembed": np.ascontiguousarray(np.asarray(inputs["atom_embed"], f32)),
        "econst": np.ascontiguousarray(ec.astype(bf16)),
        "qs_pack": np.ascontiguousarray(qs_pack),
        "wa": np.ascontiguousarray(ew1[:, :H, :]),
        "wb": np.ascontiguousarray(ew1[:, H : 2 * H, :]),
        "wcp": np.ascontiguousarray(wcp.astype(bf16)),
        "wn1a": np.ascontiguousarray(np.asarray(inputs["node_w1"], f32)[:, :H, :]),
        "w2w1b": np.ascontiguousarray(w2w1b),
        "wn2": np.ascontiguousarray(np.asarray(inputs["node_w2"], f32)),
        "brows": np.ascontiguousarray(brows),  # bf16
        "gw1": np.ascontiguousarray(np.asarray(inputs["graph_w1"], f32) / NA),
        "gw2": np.ascontiguousarray(np.asarray(inputs["graph_w2"], f32)),
        "gbrow": np.ascontiguousarray(gbrow),  # bf16
    }

    in_maps = []
    for c in range(NCORES):
        sl = slice(c * NPC, (c + 1) * NPC)
        mm = dict(shared)
        mm["coords_t"] = np.ascontiguousarray(coords[sl].T)
        mm["atypes"] = np.ascontiguousarray(atom_types[sl])
        in_maps.append(mm)
    return in_maps


_CACHE = {}
PHASE_MARKS = []
DEBUG_DUMPS = False


def _mark(nc, name):
    PHASE_MARKS.append((name, nc.next_id()))


def kernel(**inputs):
    from concourse import bass_utils

    if "nc" not in _CACHE:
        _CACHE["nc"] = build_module()
    nc = _CACHE["nc"]
    in_maps = prep_inputs(inputs)
    res = bass_utils.run_bass_kernel_spmd(
        nc, in_maps, core_ids=list(range(NCORES))
    )
    lat = np.concatenate(
        [res.results[c]["lat"].T for c in range(NCORES)], axis=0
    )  # (128, 512)
    mu, logvar = lat[:, : LAT2 // 2], lat[:, LAT2 // 2 :]
    return (mu, logvar)
